# revision 1
# baseline (speedup 1.0000x reference)
"""Trainium2 Bass kernel for nn_Loss_34230889349355 (superquadric fitting loss).

Sharding: data-parallel over batch B=8, one batch per NeuronCore.  Per core the
dominant work is the [P,S,N]=[16,200,4096] squared-distance tensor reduced by
min over S.  Distances are computed in WORLD frame (rotate is orthonormal) via
K=5 fp32r matmuls whose lhs rows are [x,y,z,1,||x||^2] and rhs rows are
[-2X', ||X'||^2, 1], so PSUM holds the full squared distance (>= 0) and no
post-min bias/relu pass is needed.

Small per-primitive tensors (superquadric surface samples, rotations, fused
rhs) are precomputed on the host - they are O(P*S) and feed the device as
plain DMA inputs, removing the trig/pow sampling stage and its activation
table loads entirely.

min over S per tile (16p x 200s = 3200 PSUM f32), engine-balanced under the
hardware rules (PSUM readable only by ACT, and by DVE with one PSUM operand
per instruction; GPSIMD cannot execute generic tensor ops in this flow):
  ACT copies h0 (8p, full depth) and the s-hi half of h1 to fp16 SBUF;
  DVE pair-mins h1's s-lo (PSUM) against the copies, then runs the packed
  fp16 2x fold chain 200->100->50->25 and a final 25-deep min-reduce.
Cuboid loss: primitive-frame coords from the same K=5 matmuls (rhs = packed
rotations) in PLANAR (xxx..yyy..zzz) layout so every select-layer op is
packed fp16 (2x DVE); squares/abs/sign/relu on ACT, axis-sum as two adds.
Existence/sparsity only need assign column sums; those plus the two weighted
partial sums ship to the host ([128,18] per core); final scalar combine in
float64 on host.
"""

import numpy as np

B, N, P, S = 8, 4096, 16, 200
T = N // 128            # 32 n-tiles
PS = P * S              # 3200 D-columns

_CACHE = {}


def _build():
    import concourse.bacc as bacc
    import concourse.tile as tile
    import concourse.bass as bass
    from concourse import mybir

    f32 = mybir.dt.float32
    f32r = mybir.dt.float32r
    f16 = mybir.dt.float16
    ALU = mybir.AluOpType
    ACT = mybir.ActivationFunctionType
    AX = mybir.AxisListType

    nc = bacc.Bacc(
        trn_type="TRN2",
        target_bir_lowering=False,
        debug=False,
        enable_asserts=False,
        num_devices=8,
    )

    bf16 = mybir.dt.bfloat16
    pc5_d = nc.dram_tensor("pc5", [5, N], f32, kind="ExternalInput")
    nr5_d = nc.dram_tensor("nr5", [5, N], f32, kind="ExternalInput")
    pc5b_d = nc.dram_tensor("pc5b", [5, N], bf16, kind="ExternalInput")
    nr5b_d = nc.dram_tensor("nr5b", [5, N], bf16, kind="ExternalInput")
    r5_d = nc.dram_tensor("r5", [5, P * 3], mybir.dt.bfloat16, kind="ExternalInput")
    rhs5_d = nc.dram_tensor("rhs5", [5, PS], f32, kind="ExternalInput")
    scl_d = nc.dram_tensor("scl16", [P * 3], f16, kind="ExternalInput")
    gout_d = nc.dram_tensor("gout", [128, T * P * 25], f16, kind="ExternalOutput")
    cubout_d = nc.dram_tensor("cubout", [128, T * P], f16, kind="ExternalOutput")

    def dap(tns, ap, offset=0):
        return bass.AP(tensor=tns, offset=offset, ap=ap)

    with tile.TileContext(nc) as tc:
        with (
            tc.tile_pool(name="consts", bufs=1) as cp,
            tc.tile_pool(name="cub", bufs=1) as bp,
            tc.tile_pool(name="wc", bufs=8) as wcp,
            tc.tile_pool(name="ww", bufs=8) as wwp,
            tc.tile_pool(name="wh", bufs=8) as whp,
            tc.tile_pool(name="wg", bufs=8) as wgp,
            tc.tile_pool(name="psum", bufs=1, space="PSUM") as pp,
            tc.tile_pool(name="psumh", bufs=1, space="PSUM") as pph,
            tc.tile_pool(name="psuml", bufs=1, space="PSUM") as ppl,
        ):
            # const AP for activation bias 0.0
            czero = cp.tile([128, 1], f32)
            nc.vector.memset(czero, 0.0)
            nc.const_aps.aps[(f32, 0.0)] = czero

            # ------------- input loads ------------------------------------
            # The [5, N] matmul operands move at per-partition DMA bandwidth,
            # so they are split into separately-tiled pieces, ordered by when
            # the main loop needs them, across the SP and GPSIMD DMA queues.
            # deps are tile-granular: every piece is its own tile.
            def g_dma(out, in_):
                nc.gpsimd.dma_start(out=out, in_=in_)

            rhsBhi = cp.tile([5, 800], f32r)
            nc.sync.dma_start(out=rhsBhi, in_=dap(rhs5_d, [[PS, 5], [1, 800]], offset=1600).bitcast(f32r))
            rhsBlo = cp.tile([5, 800], f32r)
            nc.sync.dma_start(out=rhsBlo, in_=dap(rhs5_d, [[PS, 5], [1, 800]], offset=2400).bitcast(f32r))
            pc5t01 = cp.tile([5, 256], f32r)
            g_dma(pc5t01, dap(pc5_d, [[N, 5], [1, 256]]).bitcast(f32r))
            rhsA = cp.tile([5, 1600], f32r)
            g_dma(rhsA, dap(rhs5_d, [[PS, 5], [1, 1600]]).bitcast(f32r))
            pc5A1 = cp.tile([5, 1024], f32r)
            nc.sync.dma_start(out=pc5A1, in_=dap(pc5_d, [[N, 5], [1, 1024]]).bitcast(f32r))
            pc5A2 = cp.tile([5, 1024], f32r)
            nc.sync.dma_start(out=pc5A2, in_=dap(pc5_d, [[N, 5], [1, 1024]], offset=1024).bitcast(f32r))
            pc5B = cp.tile([5, 2048], f32r)
            nc.sync.dma_start(out=pc5B, in_=dap(pc5_d, [[N, 5], [1, 2048]], offset=2048).bitcast(f32r))
            # bf16 copies for the transform matmuls: bf16 lhs streams at
            # 1 cycle/row (vs f32r's 4x penalty below 256 cols)
            pc5bA = cp.tile([5, 2048], bf16)
            g_dma(pc5bA, dap(pc5b_d, [[N, 5], [1, 2048]]))
            nr5bA = cp.tile([5, 2048], bf16)
            g_dma(nr5bA, dap(nr5b_d, [[N, 5], [1, 2048]]))
            R5f = cp.tile([5, P * 3], bf16)
            g_dma(R5f, r5_d.ap())
            pc5bB = cp.tile([5, 2048], bf16)
            nc.sync.dma_start(out=pc5bB, in_=dap(pc5b_d, [[N, 5], [1, 2048]], offset=2048))
            nr5bB = cp.tile([5, 2048], bf16)
            nc.sync.dma_start(out=nr5bB, in_=dap(nr5b_d, [[N, 5], [1, 2048]], offset=2048))
            scaleb3 = cp.tile([128, T, P * 3], f16)
            g_dma(scaleb3, dap(scl_d, [[0, 128], [0, T], [1, P * 3]]))

            def lhs_pc_of(t):
                if t < 2:
                    return pc5t01[:, 128 * t: 128 * (t + 1)]
                if t < 8:
                    return pc5A1[:, 128 * t: 128 * (t + 1)]
                if t < 16:
                    return pc5A2[:, 128 * (t - 8): 128 * (t - 7)]
                return pc5B[:, 128 * (t - 16): 128 * (t - 15)]

            def lhs_b_of(tt, which):
                a, b = (pc5bA, pc5bB) if which == "pc" else (nr5bA, nr5bB)
                src = a if tt < 16 else b
                o = 128 * (tt % 16)
                return src[:, o: o + 128]

            # PE warmup during the DMA wall: ~3us of dummy matmuls brings the
            # PE out of its low p-state before the first real tile.
            wlhs = cp.tile([1, 128], f32r)
            nc.vector.memset(wlhs.bitcast(f32), 0.0)
            wrhs = cp.tile([1, 512], f32r)
            nc.vector.memset(wrhs.bitcast(f32), 0.0)
            dwarm = pp.tile([128, 2048], f32, tag="dps", name="dwarm")
            for q in range(4):
                nc.tensor.matmul(dwarm[:, 512 * q: 512 * q + 400], wlhs,
                                 wrhs[:, 0:400], start=True, stop=True)

            # pcInI[:, t, 0, :] = pcI, [:, t, 1, :] = nI (planar 48 within)
            pcInI = cp.tile([128, T, 2, 48], f16)
            pcI_all = pcInI[:, :, 0:1, :].rearrange("n t b x -> n t (b x)")
            nI_all = pcInI[:, :, 1:2, :].rearrange("n t b x -> n t (b x)")
            cubdiff = cp.tile([128, T, P], f16)
            Gbig = cp.tile([128, T, P, 25], f16)

            # cuboid tensors (filled by ops interleaved into the main loop)
            apc = bp.tile([128, T, 48], f16)
            sgn = bp.tile([128, T, 48], f16)
            tA = bp.tile([128, T, 48], f16)
            u = bp.tile([128, T, 48], f16)
            r = bp.tile([128, T, 48], f16)
            v = bp.tile([128, T, 48], f16)
            m1 = bp.tile([128, T, 48], f16)
            q2 = bp.tile([128, T, 48], f16)
            w2 = bp.tile([128, T, 48], f16)
            dd = bp.tile([128, T, 48], f16)
            Ev = bp.tile([128, T, P], f16)
            c1 = bp.tile([128, T, P], mybir.dt.uint8)
            t1 = bp.tile([128, T, P], f16)
            d1s = bp.tile([128, T, P], f16)
            c2 = bp.tile([128, T, P], mybir.dt.uint8)
            dsel = bp.tile([128, T, P], f16)

            def _low(fn):
                with nc.allow_low_precision(reason="fp16 cuboid partials"):
                    fn()

            # cuboid ops keyed by the main-loop tile after which they are
            # emitted (transform copies own even tiles up to 18; one ACT
            # cuboid op per tile after that, DVE ops slotted per producers).
            cub_sched = {
                19: [lambda: nc.scalar.activation(apc, pcI_all, ACT.Abs)],
                20: [lambda: nc.scalar.activation(sgn, nI_all, ACT.Sign),
                     lambda: nc.vector.tensor_tensor(u, apc, scaleb3, ALU.subtract)],
                21: [lambda: nc.scalar.activation(tA, nI_all, ACT.Abs),
                     lambda: nc.vector.tensor_tensor(m1, sgn, pcI_all, ALU.mult)],
                22: [lambda: nc.vector.tensor_tensor(q2, m1, scaleb3, ALU.subtract),
                     lambda: nc.vector.tensor_tensor(c1, tA[:, :, 0:16], tA[:, :, 16:32], ALU.is_ge)],
                23: [lambda: nc.vector.tensor_scalar(r, u, 0.0, None, ALU.max),
                     lambda: nc.vector.tensor_tensor(t1, tA[:, :, 0:16], tA[:, :, 16:32], ALU.max)],
                24: [lambda: nc.vector.tensor_tensor(v, r, r, ALU.mult),
                     lambda: nc.vector.tensor_tensor(c2, t1, tA[:, :, 32:48], ALU.is_ge)],
                25: [lambda: nc.vector.tensor_tensor(w2, q2, q2, ALU.mult)],
                26: [lambda: nc.vector.tensor_tensor(dd, w2, v, ALU.subtract)],
                27: [lambda: _low(lambda: nc.vector.tensor_tensor(
                        Ev, v[:, :, 0:16], v[:, :, 16:32], ALU.add)),
                     lambda: nc.vector.select(d1s, c1, dd[:, :, 0:16], dd[:, :, 16:32])],
                28: [lambda: _low(lambda: nc.vector.tensor_tensor(
                        Ev, Ev, v[:, :, 32:48], ALU.add)),
                     lambda: nc.vector.select(dsel, c2, d1s, dd[:, :, 32:48])],
                29: [lambda: _low(lambda: nc.vector.tensor_tensor(cubdiff, Ev, dsel, ALU.add))],
                30: [lambda: nc.sync.dma_start(
                        out=cubout_d.ap(),
                        in_=cubdiff.rearrange("n t p -> n (t p)"))],
            }

            # ------------- main loop (transforms + cuboid interleaved) -----
            # Transforms for group g=(t-4)//2 ride in the unused 112-col tails
            # of even tile t's d0 PSUM quadrants (cols 400:496 of each
            # 512-block): no extra PSUM tenant, no double-buffer stall.
            def emit_folds(tf, Wf, Cf):
                # DVE: fold C -> depth 100 (packed fp16, 2x)
                nc.vector.tensor_tensor(Wf[:, 0:8, :], Cf[:, :, 0:100],
                                        Cf[:, :, 100:200], ALU.min)
                H = whp.tile([128, 16, 50], f16, tag="H", name="H")
                nc.vector.tensor_tensor(H, Wf[:, :, 0:50], Wf[:, :, 50:100], ALU.min)
                nc.vector.tensor_tensor(Gbig[:, tf, :, :], H[:, :, 0:25], H[:, :, 25:50], ALU.min)
                nc.sync.dma_start(out=dap(gout_d, [[T * P * 25, 128], [1, P * 25]],
                                          offset=P * 25 * tf),
                                  in_=Gbig[:, tf, :, :])

            prev = []
            for t in range(T):
                lhs_pc = lhs_pc_of(t)
                # h1's s-hi block first: Chi waits only these two matmuls
                d1h = pph.tile([128, 1024], f32, tag="d1h", name="d1h")
                d1hv = d1h.rearrange("n (a x) -> n a x", a=2)
                for q in range(2):
                    nc.tensor.matmul(d1hv[:, q, 0:400], lhs_pc,
                                     rhsBhi[:, 400 * q: 400 * (q + 1)],
                                     start=True, stop=True)
                d1l = ppl.tile([128, 1024], f32, tag="d1l", name="d1l")
                d1lv = d1l.rearrange("n (a x) -> n a x", a=2)
                for q in range(2):
                    nc.tensor.matmul(d1lv[:, q, 0:400], lhs_pc,
                                     rhsBlo[:, 400 * q: 400 * (q + 1)],
                                     start=True, stop=True)
                d0 = pp.tile([128, 2048], f32, tag="dps", name="d0")
                d0v = d0.rearrange("n (a x) -> n a x", a=4)
                for q in range(4):
                    nc.tensor.matmul(d0v[:, q, 0:400], lhs_pc,
                                     rhsA[:, 400 * q: 400 * (q + 1)],
                                     start=True, stop=True)
                if 4 <= t < 20 and t % 2 == 0:
                    g = (t - 4) // 2
                    for i in range(4):
                        tt = 4 * g + i
                        nc.tensor.matmul(d0v[:, i, 400:448], lhs_b_of(tt, "pc"),
                                         R5f, start=True, stop=True)
                        nc.tensor.matmul(d0v[:, i, 448:496], lhs_b_of(tt, "nr"),
                                         R5f, start=True, stop=True)

                # ACT: h1's s-hi block first (only 2 matmuls gate it)
                Chi = wcp.tile([128, 2, 4, 100], f16, tag="Chi", name="Chi")
                nc.scalar.copy(Chi, d1hv[:, :, 0:400].rearrange("n q (p s) -> n q p s", p=4))
                # ACT: evacuate h0 (8p full depth) to fp16
                C = wcp.tile([128, 8, 200], f16, tag="C", name="C")
                nc.scalar.copy(C.rearrange("n (a p) s -> n a (p s)", a=4),
                               d0v[:, :, 0:400])
                if 4 <= t < 20 and t % 2 == 0:
                    # single strided copy grabs the transform tails (pcI+nI)
                    g = (t - 4) // 2
                    nc.scalar.copy(
                        pcInI[:, 4 * g: 4 * g + 4, :, :],
                        d0v[:, :, 400:496].rearrange("n a (b x) -> n a b x", b=2))

                W = wwp.tile([128, 16, 100], f16, tag="W", name="W")
                # DVE: pair-min h1 s-lo (PSUM) vs the s-hi copies; releases d1l
                nc.vector.tensor_tensor(
                    W[:, 8:16, :].rearrange("n (q p) s -> n q p s", q=2),
                    d1lv[:, :, 0:400].rearrange("n q (p s) -> n q p s", p=4), Chi, ALU.min)
                prev.append((W, C))

                # fold chain runs one tile behind: all its inputs (previous
                # tile's C and W) are long since ready, so DVE never waits on
                # the current tile's ACT copy.
                if len(prev) > 1:
                    Wp, Cp = prev.pop(0)
                    emit_folds(t - 1, Wp, Cp)

                for op in cub_sched.get(t, []):
                    op()
            Wp, Cp = prev.pop(0)
            emit_folds(T - 1, Wp, Cp)

            # (min over depth 25, relu, A-weighted sums, and the assign
            # column sums all happen on the host from gout/cubout.)

    nc.compile()
    return nc


def _get_nc():
    if "nc" not in _CACHE:
        _CACHE["nc"] = _build()
    return _CACHE["nc"]


def _host_prep(inputs):
    import ml_dtypes
    """Per-batch input marshalling: superquadric surface samples, fused
    matmul operands.  All O(P*S) work."""
    f32 = np.float32
    in_maps = []
    for b in range(B):
        pc = np.asarray(inputs["pc"][b], dtype=np.float64)
        nr = np.asarray(inputs["normals"][b], dtype=np.float64)
        R = np.asarray(inputs["rotate"][b], dtype=np.float64)
        tr = np.asarray(inputs["trans"][b], dtype=np.float64)
        sc = np.asarray(inputs["scale"][b], dtype=np.float64)
        ep = np.asarray(inputs["shape_eps"][b], dtype=np.float64)
        et = np.asarray(inputs["etas"][b], dtype=np.float64)
        om = np.asarray(inputs["omegas"][b], dtype=np.float64)

        et = np.where(et == 0, 1e-6, et)
        om = np.where(om == 0, 1e-6, om)
        fexp = lambda x, p: np.sign(x) * np.abs(x) ** p
        ce, se = np.cos(et), np.sin(et)
        co, so = np.cos(om), np.sin(om)
        e1, e2 = ep[:, 0:1], ep[:, 1:2]
        x = sc[:, 0:1] * fexp(ce, e1) * fexp(co, e2)
        y = sc[:, 1:2] * fexp(ce, e1) * fexp(so, e2)
        z = sc[:, 2:3] * fexp(se, e1)
        clamp = lambda v: ((v > 0) * 2.0 - 1.0) * np.maximum(np.abs(v), 1e-6)
        X = np.stack([clamp(x), clamp(y), clamp(z)], -1)        # [P,S,3]
        Xw = np.einsum("pij,psj->psi", R, X) + tr[:, None, :]   # [P,S,3] world

        rhs5 = np.empty((5, PS), f32)
        rhs5[0:3] = (-2.0 * Xw).reshape(PS, 3).T
        rhs5[3] = (Xw ** 2).sum(-1).reshape(PS)
        rhs5[4] = 1.0
        r3 = rhs5.reshape(5, P, S)
        rhs5 = np.concatenate([r3[:, 0:8, :].reshape(5, 1600),
                               np.ascontiguousarray(r3[:, 8:16, 100:200]).reshape(5, 800),
                               np.ascontiguousarray(r3[:, 8:16, 0:100]).reshape(5, 800)],
                              axis=1)

        pc5 = np.empty((5, N), f32)
        pc5[0:3] = pc.T
        pc5[3] = 1.0
        pc5[4] = (pc ** 2).sum(-1)

        nr5 = np.empty((5, N), f32)
        nr5[0:3] = nr.T
        nr5[3] = 1.0
        nr5[4] = 0.0

        # planar rotation rhs: col = 16*i + p  ->  out pcI[:, :, 16i+p] = axis i
        r5 = np.empty((5, 3, P), f32)
        r5[0:3] = np.transpose(R, (1, 2, 0))                    # r5[j,i,p]=R[p,j,i]
        r5[3] = -np.einsum("pji,pj->ip", R, tr)                 # -(R^T t), planar
        r5[4] = 0.0

        in_maps.append({
            "pc5": pc5,
            "pc5b": pc5.astype(ml_dtypes.bfloat16),
            "nr5b": nr5.astype(ml_dtypes.bfloat16),
            "nr5": nr5,
            "r5": np.ascontiguousarray(r5.reshape(5, P * 3)).astype(np.float32).astype(ml_dtypes.bfloat16),
            "rhs5": rhs5,
            "scl16": np.ascontiguousarray(sc.T.reshape(P * 3)).astype(np.float16),
        })
    return in_maps


def kernel(**inputs):
    import concourse.bass_utils as bass_utils

    nc = _get_nc()
    in_maps = _host_prep(inputs)
    res = bass_utils.run_bass_kernel_spmd(nc, in_maps, core_ids=list(range(8)))

    cd_sums, cub_sums, colsums = [], [], []
    for b in range(B):
        A = np.asarray(inputs["assign_matrix"][b], dtype=np.float64)   # [N, P]
        # Ar[n_part, t, p] matches the device layout (point t*128+n_part)
        Ar = A.reshape(T, 128, P).transpose(1, 0, 2)
        G = np.asarray(res.results[b]["gout"], dtype=np.float64)
        minn = np.maximum(G.reshape(128, T, P, 25).min(-1), 0.0)
        cd_sums.append((minn * Ar).sum())
        cub = np.asarray(res.results[b]["cubout"], dtype=np.float64).reshape(128, T, P)
        cub_sums.append((cub * Ar).sum())
        colsums.append(A.sum(axis=0))

    cub = np.sum(cub_sums) / (B * N)
    cd = 2.0 * np.sum(cd_sums) / (B * N)
    ext_terms, sps_terms = [], []
    exist = np.asarray(inputs["exist"], dtype=np.float64)
    for b in range(B):
        gt = (colsums[b] > 24.0).astype(np.float64)
        pr = exist[b, :, 0]
        bce = -(gt * np.maximum(np.log(pr), -100.0)
                + (1 - gt) * np.maximum(np.log(1.0 - pr), -100.0))
        ext_terms.append(bce.mean())
        sps_terms.append(np.sqrt(colsums[b] / N + 0.01).mean() ** 2)
    ext = float(np.mean(ext_terms))
    sps = float(np.mean(sps_terms))
    loss = 1.0 * cub + 1.0 * cd + 0.1 * ext + 0.1 * sps
    return np.float32(loss)

